# revision 11
# baseline (speedup 1.0000x reference)
"""Causal linear attention (ELU+1 feature map) for Trainium2, 8 NeuronCores.

Sharding: core c handles batch b = c // 4 and head-group g = c % 4
(4 heads of 64 dims -> a 256-feature slice of the QKV/O projections).
Each core computes its partial O-projection output (2048, 1024); the host
sums the 4 partials per batch and adds bo.

Math (per head, chunked linear attention, chunk C=128, query-block 512):
  Qp = phi(x Wq^T + bq), Kp = phi(x Wk^T + bk), V = x Wv^T + bv
  Vaug = [V | 1]                              (65 columns)
  KV state (64, 65) accumulates Kp_c^T @ Vaug_c over chunks in PSUM
  numT(65, s) = Vaug_c^T @ masked(Kp_c Qp^T) + KV_prev^T-free inter term
  out = numT[:64] / (numT[64] + eps)          -> outT (feature-major)
  y_part = outT^T @ Wo_slice^T                (natural, streamed out)

All matmuls run as float32r (TF32-style) except PE transposes (fp32).
"""

import numpy as np

import concourse.bacc as bacc
import concourse.bass as bass
import concourse.mybir as mybir
import concourse.tile as tile
from concourse.bass import ds, ts
from concourse.bass_utils import run_bass_kernel_spmd
from concourse.masks import make_identity, make_upper_triangular

B, S, H_DIM = 2, 2048, 1024
N_HEADS, HEAD_DIM = 16, 64
EPS = 1e-6

N_CORES = 8
HPC = 4                  # heads per core
O = HPC * HEAD_DIM       # 256: per-core projection feature slice
CH = 128                 # key chunk
QB = 512                 # query block
N_CH = S // CH           # 16
N_QB = S // QB           # 4
CPB = QB // CH           # 4 chunks per query block
KI = H_DIM // 128        # 8 contraction chunks
SB = 512                 # projection s-block width
N_SB = S // SB           # 4

FP32 = mybir.dt.float32
FP32R = mybir.dt.float32r

AF = mybir.ActivationFunctionType


def _r(ap):
    return ap.bitcast(FP32R)


def _emit(tc):
    nc = tc.nc
    xqT = nc.dram_tensor("xqT", [H_DIM, S], FP32R, kind="ExternalInput").ap()
    xkT = nc.dram_tensor("xkT", [H_DIM, S], FP32R, kind="ExternalInput").ap()
    xvT = nc.dram_tensor("xvT", [H_DIM, S], FP32R, kind="ExternalInput").ap()
    wq = nc.dram_tensor("wq", [128, KI, O], FP32R, kind="ExternalInput").ap()
    wk = nc.dram_tensor("wk", [128, KI, O], FP32R, kind="ExternalInput").ap()
    wv = nc.dram_tensor("wv", [128, KI, O], FP32R, kind="ExternalInput").ap()
    wo = nc.dram_tensor("wo", [128, 2, H_DIM], FP32R, kind="ExternalInput").ap()
    bqd = nc.dram_tensor("bq", [128, 2], FP32, kind="ExternalInput").ap()
    bkd = nc.dram_tensor("bk", [128, 2], FP32, kind="ExternalInput").ap()
    bvd = nc.dram_tensor("bv", [1, O], FP32R, kind="ExternalInput").ap()
    y = nc.dram_tensor("y", [S, H_DIM], FP32, kind="ExternalOutput").ap()

    with tc.tile_pool(name="singles", bufs=1) as singles:
        _emit_body(tc, singles, xqT, xkT, xvT, wq, wk, wv, wo, bqd, bkd, bvd, y)


def _emit_body(tc, singles, xqT, xkT, xvT, wq, wk, wv, wo, bqd, bkd, bvd, y):
    nc = tc.nc
    # --- resident weights / constants -------------------------------------
    wq_s = singles.tile([128, KI, O], FP32R, tag="wq")
    wk_s = singles.tile([128, KI, O], FP32R, tag="wk")
    wv_s = singles.tile([128, KI, O], FP32R, tag="wv")
    wo_s = singles.tile([128, 2, H_DIM], FP32R, tag="wo")
    nc.sync.dma_start(wq_s[:], wq)
    nc.sync.dma_start(wk_s[:], wk)
    nc.sync.dma_start(wv_s[:], wv)
    nc.sync.dma_start(wo_s[:], wo)
    bq_s = singles.tile([128, 2], FP32, tag="bq")
    bk_s = singles.tile([128, 2], FP32, tag="bk")
    bv_s = singles.tile([1, O], FP32R, tag="bv")
    nc.sync.dma_start(bq_s[:], bqd)
    nc.sync.dma_start(bk_s[:], bkd)
    nc.sync.dma_start(bv_s[:], bvd)

    ident = singles.tile([128, 64], FP32, tag="ident")
    make_identity(nc, ident[0:64, :])
    make_identity(nc, ident[64:128, :])
    umask = singles.tile([128, 128], FP32, tag="umask")
    make_upper_triangular(nc, umask[:], val=1.0, diag=True)
    ones = singles.tile([1, 128], FP32R, tag="ones")
    nc.gpsimd.memset(ones[:].bitcast(FP32), 1.0)

    # --- resident activations ---------------------------------------------
    # QpT/KpT: feature-major phi'd projections; tile mt holds heads 2mt,2mt+1.
    qpt = [singles.tile([128, S], FP32R, tag=f"qpt{m}", name=f"qpt{m}") for m in range(2)]
    kpt = [singles.tile([128, S], FP32R, tag=f"kpt{m}", name=f"kpt{m}") for m in range(2)]
    # V (natural) + ones column, per chunk and head: [s128, chunk, head, 65]
    vst = singles.tile([128, N_CH, HPC, 65], FP32R, tag="vst")
    nc.gpsimd.memset(vst[:, :, :, 64:65].bitcast(FP32), 1.0)
    # outT: feature-major attention output, pair ct holds heads 2ct,2ct+1.
    outt = [singles.tile([128, S], FP32R, tag=f"outt{c}", name=f"outt{c}") for c in range(2)]

    # ======================= Phase A: projections =========================
    with (
        tc.tile_pool(name="xs", bufs=6) as xs_pool,
        tc.tile_pool(name="phi", bufs=4) as phi_pool,
        tc.tile_pool(name="pq", bufs=1, space="PSUM") as pq_pool,
        tc.tile_pool(name="pk", bufs=1, space="PSUM") as pk_pool,
        tc.tile_pool(name="pv", bufs=1, space="PSUM") as pv_pool,
    ):
        for sb in range(N_SB):
            scol = ds(sb * SB, SB)
            p_q = [pq_pool.tile([128, SB], FP32, tag=f"q{m}", name=f"pq{m}") for m in range(2)]
            p_k = [pk_pool.tile([128, SB], FP32, tag=f"k{m}", name=f"pk{m}") for m in range(2)]
            p_v = [pv_pool.tile([128, O], FP32, tag=f"v{st}", name=f"pv{st}") for st in range(4)]
            for ic in range(KI):
                xq_t = xs_pool.tile([128, SB], FP32R, tag="xq")
                nc.sync.dma_start(xq_t[:], xqT[ds(ic * 128, 128), scol])
                xk_t = xs_pool.tile([128, SB], FP32R, tag="xk")
                nc.sync.dma_start(xk_t[:], xkT[ds(ic * 128, 128), scol])
                xv_t = xs_pool.tile([128, SB], FP32R, tag="xv")
                nc.sync.dma_start(xv_t[:], xvT[ds(ic * 128, 128), scol])
                st0 = ic == 0
                for m in range(2):
                    nc.tensor.matmul(
                        p_q[m][:], wq_s[:, ic, ts(m, 128)], xq_t[:],
                        start=st0, stop=(ic == KI - 1),
                    )
                    nc.tensor.matmul(
                        p_k[m][:], wk_s[:, ic, ts(m, 128)], xk_t[:],
                        start=st0, stop=(ic == KI - 1),
                    )
                for st in range(4):
                    nc.tensor.matmul(
                        p_v[st][:], xv_t[:, ts(st, 128)], wv_s[:, ic, :],
                        start=st0, stop=False,
                    )
            # v bias via K=1 ones-column matmul, closes the group
            for st in range(4):
                nc.tensor.matmul(
                    p_v[st][:], ones[:, 0:128], bv_s[:],
                    start=False, stop=True,
                )
            # phi( q ), phi( k ): relu(x+b) + min(exp(x+b), 1)
            for m in range(2):
                for p_x, b_x, dst in ((p_q[m], bq_s, qpt[m]), (p_k[m], bk_s, kpt[m])):
                    e_t = phi_pool.tile([128, SB], FP32, tag="e")
                    nc.scalar.activation(e_t[:], p_x[:], AF.Exp, bias=b_x[:, ds(m, 1)])
                    nc.scalar.activation(
                        dst[:, scol], p_x[:], AF.Relu, bias=b_x[:, ds(m, 1)]
                    )
                    nc.gpsimd.tensor_scalar_min(e_t[:], e_t[:], 1.0)
                    nc.vector.tensor_add(dst[:, scol], dst[:, scol], e_t[:])
            # v -> vstore (+ ones column preset at init)
            for st in range(4):
                c = sb * 4 + st
                nc.vector.tensor_copy(
                    vst[:, c, :, 0:64],
                    p_v[st][:].rearrange("p (h d) -> p h d", h=HPC),
                )

    # ================= Phase B + C: attention + O-projection ==============
    kv_sb = [
        singles.tile([128, 65], FP32R, tag=f"kvsb{h}", name=f"kvsb{h}")
        for h in range(HPC)
    ]
    for h in range(HPC):
        nc.gpsimd.memset(kv_sb[h][:].bitcast(FP32), 0.0)

    with (
        tc.tile_pool(name="pnum", bufs=2, space="PSUM") as pnum_pool,
        tc.tile_pool(name="pbig", bufs=4, space="PSUM") as pbig_pool,
        tc.tile_pool(name="ssb", bufs=4) as ssb_pool,
        tc.tile_pool(name="knb", bufs=4) as kn_pool,
        tc.tile_pool(name="den", bufs=4) as den_pool,
        tc.tile_pool(name="yt", bufs=2) as yt_pool,
    ):
        for qb in range(N_QB):
            for h in range(HPC):
                mt, prow = h // 2, 64 * (h % 2)
                qp_h = qpt[mt][ds(prow, 64), :]
                kp_h = kpt[mt][ds(prow, 64), :]
                p_num = pnum_pool.tile([65, QB], FP32, tag="num")
                started = False
                if qb > 0:
                    nc.tensor.matmul(
                        p_num[:], kv_sb[h][ds(prow, 64), :],
                        qp_h[:, ds(qb * QB, QB)],
                        start=True, stop=False,
                    )
                    started = True
                p_kv = None
                if qb < N_QB - 1:
                    p_kv = pbig_pool.tile([64, 65], FP32, tag="big", name=f"pkv{h}")
                for cj in range(CPB):
                    c = qb * CPB + cj
                    nq = QB - cj * CH
                    qoff = qb * QB + cj * CH
                    # S^T for chunk c against remaining queries of the block
                    p_s = pbig_pool.tile([128, nq], FP32, tag="big")
                    nc.tensor.matmul(
                        p_s[:], kp_h[:, ds(c * CH, CH)], qp_h[:, ds(qoff, nq)],
                        start=True, stop=True,
                    )
                    s_t = ssb_pool.tile([128, nq], FP32R, tag="s")
                    nc.vector.tensor_mul(s_t[:, 0:CH], p_s[:, 0:CH], umask[:])
                    if nq > CH:
                        nc.scalar.copy(s_t[:, CH:nq], p_s[:, CH:nq])
                    # numerator (+denominator via ones column)
                    nc.tensor.matmul(
                        p_num[:, ds(cj * CH, nq)], vst[:, c, h, :], s_t[:],
                        start=not started, stop=(cj == CPB - 1),
                    )
                    started = True
                    # KV state update (skipped for the last block: never read)
                    if p_kv is not None:
                        p_t = pbig_pool.tile([128, 64], FP32, tag="big")
                        nc.tensor.transpose(
                            p_t[:], kp_h[:, ds(c * CH, CH)].bitcast(FP32), ident[ds(prow, 64), :]
                        )
                        kn_t = kn_pool.tile([128, 64], FP32, tag="kn")
                        nc.vector.tensor_copy(kn_t[:], p_t[:])
                        nc.tensor.matmul(
                            p_kv[:], kn_t[:], vst[:, c, h, :].bitcast(FP32),
                            start=(cj == 0), stop=(cj == CPB - 1),
                        )
                if p_kv is not None:
                    nc.vector.tensor_add(
                        kv_sb[h][ds(prow, 64), :], kv_sb[h][ds(prow, 64), :], p_kv[:]
                    )
                # divide: outT = num / (den + eps)
                den_t = den_pool.tile([1, QB], FP32, tag="den")
                nc.vector.tensor_scalar_add(den_t[:], p_num[ds(64, 1), :], EPS)
                nc.vector.reciprocal(den_t[:], den_t[:])
                bc_t = den_pool.tile([64, QB], FP32, tag="bc")
                nc.gpsimd.partition_broadcast(bc_t[:], den_t[:])
                nc.vector.tensor_mul(
                    outt[mt][ds(prow, 64), ds(qb * QB, QB)], p_num[0:64, :], bc_t[:]
                )
            # O-projection for the 4 s-tiles of this query block
            for st in range(qb * CPB, (qb + 1) * CPB):
                y_t = yt_pool.tile([128, H_DIM], FP32, tag="y")
                for n in range(2):
                    p_o = pbig_pool.tile([128, 512], FP32, tag="big")
                    for ct in range(2):
                        nc.tensor.matmul(
                            p_o[:], outt[ct][:, ts(st, 128)],
                            wo_s[:, ct, ts(n, 512)],
                            start=(ct == 0), stop=(ct == 1),
                        )
                    nc.scalar.copy(y_t[:, ts(n, 512)], p_o[:])
                nc.sync.dma_start(y[ds(st * 128, 128), :], y_t[:])


_PROGRAM = None


def _get_program():
    global _PROGRAM
    if _PROGRAM is None:
        nc = bacc.Bacc("TRN2", target_bir_lowering=False, debug=False)
        with tile.TileContext(nc) as tc:
            _emit(tc)
        nc.compile()
        _PROGRAM = nc
    return _PROGRAM


def kernel(query, key, value, Wq, bq, Wk, bk, Wv, bv, Wo, bo, _trace=False):
    query, key, value = (np.asarray(a, np.float32) for a in (query, key, value))
    Wq, Wk, Wv, Wo = (np.asarray(a, np.float32) for a in (Wq, Wk, Wv, Wo))
    bq, bk, bv, bo = (np.asarray(a, np.float32) for a in (bq, bk, bv, bo))

    def wslice(W, g):  # (1024, 256) -> (128, 8, 256) contraction-chunked
        wt = W[g * O:(g + 1) * O].T  # (1024, 256)
        return np.ascontiguousarray(wt.reshape(KI, 128, O).transpose(1, 0, 2))

    in_maps = []
    for c in range(N_CORES):
        b, g = divmod(c, 4)
        sl = slice(g * O, (g + 1) * O)
        in_maps.append({
            "xqT": np.ascontiguousarray(query[b].T),
            "xkT": np.ascontiguousarray(key[b].T),
            "xvT": np.ascontiguousarray(value[b].T),
            "wq": wslice(Wq, g),
            "wk": wslice(Wk, g),
            "wv": wslice(Wv, g),
            "wo": np.ascontiguousarray(
                Wo[:, sl].T.reshape(2, 128, H_DIM).transpose(1, 0, 2)
            ),
            "bq": np.ascontiguousarray(bq[sl].reshape(2, 128).T),
            "bk": np.ascontiguousarray(bk[sl].reshape(2, 128).T),
            "bv": np.ascontiguousarray(bv[sl].reshape(1, O)),
        })

    nc = _get_program()
    res = run_bass_kernel_spmd(
        nc, in_maps, core_ids=list(range(N_CORES)), trace=_trace
    )
    out = np.empty((B, S, H_DIM), np.float32)
    for b in range(B):
        acc = res.results[4 * b]["y"].astype(np.float64)
        for g in range(1, 4):
            acc += res.results[4 * b + g]["y"]
        out[b] = (acc + bo).astype(np.float32)
    if _trace:
        kernel.last_result = res
    return out


# revision 13
# speedup vs baseline: 1.3270x; 1.3270x over previous
"""Causal linear attention (ELU+1 feature map) for Trainium2, 8 NeuronCores.

Sharding: core c handles batch b = c // 4 and head-group g = c % 4
(4 heads of 64 dims -> a 256-feature slice of the QKV/O projections).
Each core computes its partial O-projection output (2048, 1024); the host
sums the 4 partials per batch and adds bo.

Math (per head, chunked linear attention, chunk C=128, query-block 512):
  Qp = phi(x Wq^T + bq), Kp = phi(x Wk^T + bk), V = x Wv^T + bv
  Vaug = [V | 1]                              (65 columns)
  KV state (64, 65) accumulates Kp_c^T @ Vaug_c over chunks in PSUM
  numT(65, s) = Vaug_c^T @ masked(Kp_c Qp^T) + KV_prev^T-free inter term
  out = numT[:64] / (numT[64] + eps)          -> outT (feature-major)
  y_part = outT^T @ Wo_slice^T                (natural, streamed out)

All matmuls run as float32r (TF32-style) except PE transposes (fp32).
"""

import numpy as np

import concourse.bacc as bacc
import concourse.bass as bass
import concourse.mybir as mybir
import concourse.tile as tile
from concourse.bass import ds, ts
from concourse.bass_utils import run_bass_kernel_spmd
from concourse.masks import make_identity, make_upper_triangular

B, S, H_DIM = 2, 2048, 1024
N_HEADS, HEAD_DIM = 16, 64
EPS = 1e-6

N_CORES = 8
HPC = 4                  # heads per core
O = HPC * HEAD_DIM       # 256: per-core projection feature slice
CH = 128                 # key chunk
QB = 512                 # query block
N_CH = S // CH           # 16
N_QB = S // QB           # 4
CPB = QB // CH           # 4 chunks per query block
KI = H_DIM // 128        # 8 contraction chunks
SB = 512                 # projection s-block width
N_SB = S // SB           # 4

FP32 = mybir.dt.float32
FP32R = mybir.dt.float32r

AF = mybir.ActivationFunctionType


def _r(ap):
    return ap.bitcast(FP32R)


def _emit(tc):
    nc = tc.nc
    xqT = nc.dram_tensor("xqT", [H_DIM, S], FP32R, kind="ExternalInput").ap()
    xkT = nc.dram_tensor("xkT", [H_DIM, S], FP32R, kind="ExternalInput").ap()
    xvT = nc.dram_tensor("xvT", [H_DIM, S], FP32R, kind="ExternalInput").ap()
    wq = nc.dram_tensor("wq", [128, KI, O], FP32R, kind="ExternalInput").ap()
    wk = nc.dram_tensor("wk", [128, KI, O], FP32R, kind="ExternalInput").ap()
    wv = nc.dram_tensor("wv", [128, KI, O], FP32R, kind="ExternalInput").ap()
    wo = nc.dram_tensor("wo", [128, 2, H_DIM], FP32R, kind="ExternalInput").ap()
    bqd = nc.dram_tensor("bq", [128, 2], FP32, kind="ExternalInput").ap()
    bkd = nc.dram_tensor("bk", [128, 2], FP32, kind="ExternalInput").ap()
    bvd = nc.dram_tensor("bv", [1, O], FP32R, kind="ExternalInput").ap()
    y = nc.dram_tensor("y", [S, H_DIM], FP32, kind="ExternalOutput").ap()

    with tc.tile_pool(name="singles", bufs=1) as singles:
        _emit_body(tc, singles, xqT, xkT, xvT, wq, wk, wv, wo, bqd, bkd, bvd, y)


def _emit_body(tc, singles, xqT, xkT, xvT, wq, wk, wv, wo, bqd, bkd, bvd, y):
    nc = tc.nc
    # --- resident weights / constants -------------------------------------
    wq_s = singles.tile([128, KI, O], FP32R, tag="wq")
    wk_s = singles.tile([128, KI, O], FP32R, tag="wk")
    wv_s = singles.tile([128, KI, O], FP32R, tag="wv")
    wo_s = singles.tile([128, 2, H_DIM], FP32R, tag="wo")
    nc.sync.dma_start(wq_s[:], wq)
    nc.sync.dma_start(wk_s[:], wk)
    nc.sync.dma_start(wv_s[:], wv)
    nc.sync.dma_start(wo_s[:], wo)
    bq_s = singles.tile([128, 2], FP32, tag="bq")
    bk_s = singles.tile([128, 2], FP32, tag="bk")
    bv_s = singles.tile([1, O], FP32R, tag="bv")
    nc.sync.dma_start(bq_s[:], bqd)
    nc.sync.dma_start(bk_s[:], bkd)
    nc.sync.dma_start(bv_s[:], bvd)

    ident = singles.tile([128, 64], FP32, tag="ident")
    make_identity(nc, ident[0:64, :])
    make_identity(nc, ident[64:128, :])
    umask = singles.tile([128, 128], FP32, tag="umask")
    make_upper_triangular(nc, umask[:], val=1.0, diag=True)
    ones = singles.tile([1, 128], FP32R, tag="ones")
    nc.gpsimd.memset(ones[:].bitcast(FP32), 1.0)

    # --- resident activations ---------------------------------------------
    # QpT/KpT: feature-major phi'd projections; tile mt holds heads 2mt,2mt+1.
    qpt = [singles.tile([128, S], FP32R, tag=f"qpt{m}", name=f"qpt{m}") for m in range(2)]
    kpt = [singles.tile([128, S], FP32R, tag=f"kpt{m}", name=f"kpt{m}") for m in range(2)]
    # V (natural) + ones column, per chunk and head: [s128, chunk, head, 65]
    vst = singles.tile([128, N_CH, HPC, 65], FP32R, tag="vst")
    nc.gpsimd.memset(vst[:, :, :, 64:65].bitcast(FP32), 1.0)
    # outT: feature-major attention output, pair ct holds heads 2ct,2ct+1.
    outt = [singles.tile([128, S], FP32R, tag=f"outt{c}", name=f"outt{c}") for c in range(2)]

    # ======================= Phase A: projections =========================
    with (
        tc.tile_pool(name="xs", bufs=6) as xs_pool,
        tc.tile_pool(name="phi", bufs=4) as phi_pool,
        tc.tile_pool(name="pq", bufs=1, space="PSUM") as pq_pool,
        tc.tile_pool(name="pk", bufs=1, space="PSUM") as pk_pool,
        tc.tile_pool(name="pv", bufs=1, space="PSUM") as pv_pool,
    ):
        for sb in range(N_SB):
            scol = ds(sb * SB, SB)
            p_q = [pq_pool.tile([128, SB], FP32, tag=f"q{m}", name=f"pq{m}") for m in range(2)]
            p_k = [pk_pool.tile([128, SB], FP32, tag=f"k{m}", name=f"pk{m}") for m in range(2)]
            p_v = [pv_pool.tile([128, O], FP32, tag=f"v{st}", name=f"pv{st}") for st in range(4)]
            for ic in range(KI):
                xq_t = xs_pool.tile([128, SB], FP32R, tag="xq")
                nc.sync.dma_start(xq_t[:], xqT[ds(ic * 128, 128), scol])
                xk_t = xs_pool.tile([128, SB], FP32R, tag="xk")
                nc.sync.dma_start(xk_t[:], xkT[ds(ic * 128, 128), scol])
                xv_t = xs_pool.tile([128, SB], FP32R, tag="xv")
                nc.sync.dma_start(xv_t[:], xvT[ds(ic * 128, 128), scol])
                st0 = ic == 0
                for m in range(2):
                    nc.tensor.matmul(
                        p_q[m][:], wq_s[:, ic, ts(m, 128)], xq_t[:],
                        start=st0, stop=(ic == KI - 1),
                    )
                    nc.tensor.matmul(
                        p_k[m][:], wk_s[:, ic, ts(m, 128)], xk_t[:],
                        start=st0, stop=(ic == KI - 1),
                    )
                for st in range(4):
                    nc.tensor.matmul(
                        p_v[st][:], xv_t[:, ts(st, 128)], wv_s[:, ic, :],
                        start=st0, stop=False,
                    )
            # v bias via K=1 ones-column matmul, closes the group
            for st in range(4):
                nc.tensor.matmul(
                    p_v[st][:], ones[:, 0:128], bv_s[:],
                    start=False, stop=True,
                )
            # phi( q ), phi( k ): relu(x+b) + min(exp(x+b), 1)
            for m in range(2):
                for p_x, b_x, dst in ((p_q[m], bq_s, qpt[m]), (p_k[m], bk_s, kpt[m])):
                    e_t = phi_pool.tile([128, SB], FP32, tag="e")
                    nc.scalar.activation(e_t[:], p_x[:], AF.Exp, bias=b_x[:, ds(m, 1)])
                    r_t = phi_pool.tile([128, SB], FP32, tag="r")
                    nc.scalar.activation(r_t[:], p_x[:], AF.Relu, bias=b_x[:, ds(m, 1)])
                    nc.vector.tensor_scalar_min(e_t[:], e_t[:], 1.0)
                    nc.vector.tensor_add(dst[:, scol], r_t[:], e_t[:])
            # v -> vstore (+ ones column preset at init)
            for st in range(4):
                c = sb * 4 + st
                nc.vector.tensor_copy(
                    vst[:, c, :, 0:64],
                    p_v[st][:].rearrange("p (h d) -> p h d", h=HPC),
                )

    # ================= Phase B + C: attention + O-projection ==============
    kv_sb = [
        singles.tile([128, 65], FP32R, tag=f"kvsb{h}", name=f"kvsb{h}")
        for h in range(HPC)
    ]
    for h in range(HPC):
        nc.gpsimd.memset(kv_sb[h][:].bitcast(FP32), 0.0)

    with (
        tc.tile_pool(name="pnum", bufs=2, space="PSUM") as pnum_pool,
        tc.tile_pool(name="pbig", bufs=4, space="PSUM") as pbig_pool,
        tc.tile_pool(name="ssb", bufs=4) as ssb_pool,
        tc.tile_pool(name="knb", bufs=4) as kn_pool,
        tc.tile_pool(name="den", bufs=4) as den_pool,
        tc.tile_pool(name="yt", bufs=2) as yt_pool,
    ):
        for qb in range(N_QB):
            for h in range(HPC):
                mt, prow = h // 2, 64 * (h % 2)
                qp_h = qpt[mt][ds(prow, 64), :]
                kp_h = kpt[mt][ds(prow, 64), :]
                p_num = pnum_pool.tile([65, QB], FP32, tag="num")
                started = False
                if qb > 0:
                    nc.tensor.matmul(
                        p_num[:], kv_sb[h][ds(prow, 64), :],
                        qp_h[:, ds(qb * QB, QB)],
                        start=True, stop=False,
                    )
                    started = True
                p_kv = None
                if qb < N_QB - 1:
                    p_kv = pbig_pool.tile([64, 65], FP32, tag="big", name=f"pkv{h}")
                for cj in range(CPB):
                    c = qb * CPB + cj
                    nq = QB - cj * CH
                    qoff = qb * QB + cj * CH
                    # S^T for chunk c against remaining queries of the block
                    p_s = pbig_pool.tile([128, nq], FP32, tag="big")
                    nc.tensor.matmul(
                        p_s[:], kp_h[:, ds(c * CH, CH)], qp_h[:, ds(qoff, nq)],
                        start=True, stop=True,
                    )
                    s_t = ssb_pool.tile([128, nq], FP32R, tag="s")
                    nc.vector.tensor_mul(s_t[:, 0:CH], p_s[:, 0:CH], umask[:])
                    if nq > CH:
                        nc.scalar.copy(s_t[:, CH:nq], p_s[:, CH:nq])
                    # numerator (+denominator via ones column)
                    nc.tensor.matmul(
                        p_num[:, ds(cj * CH, nq)], vst[:, c, h, :], s_t[:],
                        start=not started, stop=(cj == CPB - 1),
                    )
                    started = True
                    # KV state update (skipped for the last block: never read)
                    if p_kv is not None:
                        p_t = pbig_pool.tile([128, 64], FP32, tag="big")
                        nc.tensor.transpose(
                            p_t[:], kp_h[:, ds(c * CH, CH)].bitcast(FP32), ident[ds(prow, 64), :]
                        )
                        kn_t = kn_pool.tile([128, 64], FP32, tag="kn")
                        nc.vector.tensor_copy(kn_t[:], p_t[:])
                        nc.tensor.matmul(
                            p_kv[:], kn_t[:], vst[:, c, h, :].bitcast(FP32),
                            start=(cj == 0), stop=(cj == CPB - 1),
                        )
                if p_kv is not None:
                    nc.vector.tensor_add(
                        kv_sb[h][ds(prow, 64), :], kv_sb[h][ds(prow, 64), :], p_kv[:]
                    )
                # divide: outT = num / (den + eps)
                den_t = den_pool.tile([1, QB], FP32, tag="den")
                nc.scalar.activation(den_t[:], p_num[ds(64, 1), :], AF.Ln, bias=0.0)
                rden = den_pool.tile([1, QB], FP32, tag="rden")
                nc.scalar.activation(rden[:], den_t[:], AF.Exp, scale=-1.0)
                bc_t = den_pool.tile([64, QB], FP32, tag="bc")
                nc.gpsimd.partition_broadcast(bc_t[:], rden[:])
                nc.vector.tensor_mul(
                    outt[mt][ds(prow, 64), ds(qb * QB, QB)], p_num[0:64, :], bc_t[:]
                )
            # O-projection for the 4 s-tiles of this query block
            for st in range(qb * CPB, (qb + 1) * CPB):
                y_t = yt_pool.tile([128, H_DIM], FP32, tag="y")
                for n in range(2):
                    p_o = pbig_pool.tile([128, 512], FP32, tag="big")
                    for ct in range(2):
                        nc.tensor.matmul(
                            p_o[:], outt[ct][:, ts(st, 128)],
                            wo_s[:, ct, ts(n, 512)],
                            start=(ct == 0), stop=(ct == 1),
                        )
                    nc.scalar.copy(y_t[:, ts(n, 512)], p_o[:])
                nc.sync.dma_start(y[ds(st * 128, 128), :], y_t[:])


_PROGRAM = None


def _get_program():
    global _PROGRAM
    if _PROGRAM is None:
        nc = bacc.Bacc("TRN2", target_bir_lowering=False, debug=False)
        with tile.TileContext(nc) as tc:
            _emit(tc)
        nc.compile()
        _PROGRAM = nc
    return _PROGRAM


def kernel(query, key, value, Wq, bq, Wk, bk, Wv, bv, Wo, bo, _trace=False):
    query, key, value = (np.asarray(a, np.float32) for a in (query, key, value))
    Wq, Wk, Wv, Wo = (np.asarray(a, np.float32) for a in (Wq, Wk, Wv, Wo))
    bq, bk, bv, bo = (np.asarray(a, np.float32) for a in (bq, bk, bv, bo))

    def wslice(W, g):  # (1024, 256) -> (128, 8, 256) contraction-chunked
        wt = W[g * O:(g + 1) * O].T  # (1024, 256)
        return np.ascontiguousarray(wt.reshape(KI, 128, O).transpose(1, 0, 2))

    in_maps = []
    for c in range(N_CORES):
        b, g = divmod(c, 4)
        sl = slice(g * O, (g + 1) * O)
        in_maps.append({
            "xqT": np.ascontiguousarray(query[b].T),
            "xkT": np.ascontiguousarray(key[b].T),
            "xvT": np.ascontiguousarray(value[b].T),
            "wq": wslice(Wq, g),
            "wk": wslice(Wk, g),
            "wv": wslice(Wv, g),
            "wo": np.ascontiguousarray(
                Wo[:, sl].T.reshape(2, 128, H_DIM).transpose(1, 0, 2)
            ),
            "bq": np.ascontiguousarray(bq[sl].reshape(2, 128).T),
            "bk": np.ascontiguousarray(bk[sl].reshape(2, 128).T),
            "bv": np.ascontiguousarray(bv[sl].reshape(1, O)),
        })

    nc = _get_program()
    res = run_bass_kernel_spmd(
        nc, in_maps, core_ids=list(range(N_CORES)), trace=_trace
    )
    out = np.empty((B, S, H_DIM), np.float32)
    for b in range(B):
        acc = res.results[4 * b]["y"].astype(np.float64)
        for g in range(1, 4):
            acc += res.results[4 * b + g]["y"]
        out[b] = (acc + bo).astype(np.float32)
    if _trace:
        kernel.last_result = res
    return out


# revision 14
# speedup vs baseline: 1.4850x; 1.1190x over previous
"""Causal linear attention (ELU+1 feature map) for Trainium2, 8 NeuronCores.

Sharding: core c handles batch b = c // 4 and head-group g = c % 4
(4 heads of 64 dims -> a 256-feature slice of the QKV/O projections).
Each core computes its partial O-projection output (2048, 1024); the host
sums the 4 partials per batch and adds bo.

Math (per head, chunked linear attention, chunk C=128, query-block 512):
  Qp = phi(x Wq^T + bq), Kp = phi(x Wk^T + bk), V = x Wv^T + bv
  Vaug = [V | 1]                              (65 columns)
  KV state (64, 65) accumulates Kp_c^T @ Vaug_c over chunks in PSUM
  numT(65, s) = Vaug_c^T @ masked(Kp_c Qp^T) + KV_prev^T-free inter term
  out = numT[:64] / (numT[64] + eps)          -> outT (feature-major)
  y_part = outT^T @ Wo_slice^T                (natural, streamed out)

All matmuls run as float32r (TF32-style) except PE transposes (fp32).
"""

import numpy as np

import concourse.bacc as bacc
import concourse.bass as bass
import concourse.mybir as mybir
import concourse.tile as tile
from concourse.bass import ds, ts
from concourse.bass_utils import run_bass_kernel_spmd
from concourse.masks import make_identity, make_upper_triangular

B, S, H_DIM = 2, 2048, 1024
N_HEADS, HEAD_DIM = 16, 64
EPS = 1e-6

N_CORES = 8
HPC = 4                  # heads per core
O = HPC * HEAD_DIM       # 256: per-core projection feature slice
CH = 128                 # key chunk
QB = 512                 # query block
N_CH = S // CH           # 16
N_QB = S // QB           # 4
CPB = QB // CH           # 4 chunks per query block
KI = H_DIM // 128        # 8 contraction chunks
SB = 512                 # projection s-block width
N_SB = S // SB           # 4

FP32 = mybir.dt.float32
FP32R = mybir.dt.float32r

AF = mybir.ActivationFunctionType


def _r(ap):
    return ap.bitcast(FP32R)


def _emit(tc):
    nc = tc.nc
    xqT = nc.dram_tensor("xqT", [H_DIM, S], FP32R, kind="ExternalInput").ap()
    xkT = nc.dram_tensor("xkT", [H_DIM, S], FP32R, kind="ExternalInput").ap()
    xvT = nc.dram_tensor("xvT", [H_DIM, S], FP32R, kind="ExternalInput").ap()
    wq = nc.dram_tensor("wq", [128, KI, O], FP32R, kind="ExternalInput").ap()
    wk = nc.dram_tensor("wk", [128, KI, O], FP32R, kind="ExternalInput").ap()
    wv = nc.dram_tensor("wv", [128, KI, O], FP32R, kind="ExternalInput").ap()
    wo = nc.dram_tensor("wo", [128, 2, H_DIM], FP32R, kind="ExternalInput").ap()
    bqd = nc.dram_tensor("bq", [128, 2], FP32, kind="ExternalInput").ap()
    bkd = nc.dram_tensor("bk", [128, 2], FP32, kind="ExternalInput").ap()
    bvd = nc.dram_tensor("bv", [1, O], FP32R, kind="ExternalInput").ap()
    y = nc.dram_tensor("y", [S, H_DIM], FP32, kind="ExternalOutput").ap()

    with tc.tile_pool(name="singles", bufs=1) as singles:
        _emit_body(tc, singles, xqT, xkT, xvT, wq, wk, wv, wo, bqd, bkd, bvd, y)


def _emit_body(tc, singles, xqT, xkT, xvT, wq, wk, wv, wo, bqd, bkd, bvd, y):
    nc = tc.nc
    # --- resident weights / constants -------------------------------------
    wq_s = singles.tile([128, KI, O], FP32R, tag="wq")
    wk_s = singles.tile([128, KI, O], FP32R, tag="wk")
    wv_s = singles.tile([128, KI, O], FP32R, tag="wv")
    wo_s = singles.tile([128, 2, H_DIM], FP32R, tag="wo")
    for ic in range(KI):
        nc.gpsimd.dma_start(wq_s[:, ic, :], wq[:, ic, :])
        nc.gpsimd.dma_start(wk_s[:, ic, :], wk[:, ic, :])
        nc.gpsimd.dma_start(wv_s[:, ic, :], wv[:, ic, :])
    nc.gpsimd.dma_start(wo_s[:], wo)
    bq_s = singles.tile([128, 2], FP32, tag="bq")
    bk_s = singles.tile([128, 2], FP32, tag="bk")
    bv_s = singles.tile([1, O], FP32R, tag="bv")
    nc.gpsimd.dma_start(bq_s[:], bqd)
    nc.gpsimd.dma_start(bk_s[:], bkd)
    nc.gpsimd.dma_start(bv_s[:], bvd)

    ident = singles.tile([128, 64], FP32, tag="ident")
    make_identity(nc, ident[0:64, :])
    make_identity(nc, ident[64:128, :])
    umask = singles.tile([128, 128], FP32, tag="umask")
    make_upper_triangular(nc, umask[:], val=1.0, diag=True)
    ones = singles.tile([1, 128], FP32R, tag="ones")
    nc.gpsimd.memset(ones[:].bitcast(FP32), 1.0)

    # --- resident activations ---------------------------------------------
    # QpT/KpT: feature-major phi'd projections; tile mt holds heads 2mt,2mt+1.
    qpt = [singles.tile([128, S], FP32R, tag=f"qpt{m}", name=f"qpt{m}") for m in range(2)]
    kpt = [singles.tile([128, S], FP32R, tag=f"kpt{m}", name=f"kpt{m}") for m in range(2)]
    # V (natural) + ones column, per chunk and head: [s128, chunk, head, 65]
    vst = singles.tile([128, N_CH, HPC, 65], FP32R, tag="vst")
    nc.gpsimd.memset(vst[:, :, :, 64:65].bitcast(FP32), 1.0)
    # outT: feature-major attention output, pair ct holds heads 2ct,2ct+1.
    outt = [singles.tile([128, S], FP32R, tag=f"outt{c}", name=f"outt{c}") for c in range(2)]

    # ======================= Phase A: projections =========================
    with (
        tc.tile_pool(name="xs", bufs=6) as xs_pool,
        tc.tile_pool(name="phi", bufs=4) as phi_pool,
        tc.tile_pool(name="pq", bufs=1, space="PSUM") as pq_pool,
        tc.tile_pool(name="pk", bufs=1, space="PSUM") as pk_pool,
        tc.tile_pool(name="pv", bufs=1, space="PSUM") as pv_pool,
    ):
        for sb in range(N_SB):
            scol = ds(sb * SB, SB)
            p_q = [pq_pool.tile([128, SB], FP32, tag=f"q{m}", name=f"pq{m}") for m in range(2)]
            p_k = [pk_pool.tile([128, SB], FP32, tag=f"k{m}", name=f"pk{m}") for m in range(2)]
            p_v = [pv_pool.tile([128, O], FP32, tag=f"v{st}", name=f"pv{st}") for st in range(4)]
            for ic in range(KI):
                xq_t = xs_pool.tile([128, SB], FP32R, tag="xq")
                nc.sync.dma_start(xq_t[:], xqT[ds(ic * 128, 128), scol])
                xk_t = xs_pool.tile([128, SB], FP32R, tag="xk")
                nc.sync.dma_start(xk_t[:], xkT[ds(ic * 128, 128), scol])
                xv_t = xs_pool.tile([128, SB], FP32R, tag="xv")
                nc.sync.dma_start(xv_t[:], xvT[ds(ic * 128, 128), scol])
                st0 = ic == 0
                for m in range(2):
                    nc.tensor.matmul(
                        p_q[m][:], wq_s[:, ic, ts(m, 128)], xq_t[:],
                        start=st0, stop=(ic == KI - 1),
                    )
                    nc.tensor.matmul(
                        p_k[m][:], wk_s[:, ic, ts(m, 128)], xk_t[:],
                        start=st0, stop=(ic == KI - 1),
                    )
                for st in range(4):
                    nc.tensor.matmul(
                        p_v[st][:], xv_t[:, ts(st, 128)], wv_s[:, ic, :],
                        start=st0, stop=False,
                    )
            # v bias via K=1 ones-column matmul, closes the group
            for st in range(4):
                nc.tensor.matmul(
                    p_v[st][:], ones[:, 0:128], bv_s[:],
                    start=False, stop=True,
                )
            # phi( q ), phi( k ): relu(x+b) + min(exp(x+b), 1)
            for m in range(2):
                for p_x, b_x, dst in ((p_q[m], bq_s, qpt[m]), (p_k[m], bk_s, kpt[m])):
                    e_t = phi_pool.tile([128, SB], FP32, tag="e")
                    nc.scalar.activation(e_t[:], p_x[:], AF.Exp, bias=b_x[:, ds(m, 1)])
                    r_t = phi_pool.tile([128, SB], FP32, tag="r")
                    nc.scalar.activation(r_t[:], p_x[:], AF.Relu, bias=b_x[:, ds(m, 1)])
                    nc.vector.tensor_scalar_min(e_t[:], e_t[:], 1.0)
                    nc.vector.tensor_add(dst[:, scol], r_t[:], e_t[:])
            # v -> vstore (+ ones column preset at init)
            for st in range(4):
                c = sb * 4 + st
                nc.vector.tensor_copy(
                    vst[:, c, :, 0:64],
                    p_v[st][:].rearrange("p (h d) -> p h d", h=HPC),
                )

    # ================= Phase B + C: attention + O-projection ==============
    kv_sb = [
        singles.tile([128, 65], FP32R, tag=f"kvsb{h}", name=f"kvsb{h}")
        for h in range(HPC)
    ]
    for h in range(HPC):
        nc.gpsimd.memset(kv_sb[h][:].bitcast(FP32), 0.0)

    with (
        tc.tile_pool(name="pnum", bufs=3, space="PSUM") as pnum_pool,
        tc.tile_pool(name="pbig", bufs=5, space="PSUM") as pbig_pool,
        tc.tile_pool(name="ssb", bufs=6) as ssb_pool,
        tc.tile_pool(name="knb", bufs=6) as kn_pool,
        tc.tile_pool(name="den", bufs=8) as den_pool,
        tc.tile_pool(name="yt", bufs=3) as yt_pool,
    ):
        for qb in range(N_QB):
            for h in range(HPC):
                mt, prow = h // 2, 64 * (h % 2)
                qp_h = qpt[mt][ds(prow, 64), :]
                kp_h = kpt[mt][ds(prow, 64), :]
                p_num = pnum_pool.tile([65, QB], FP32, tag="num")
                started = False
                if qb > 0:
                    nc.tensor.matmul(
                        p_num[:], kv_sb[h][ds(prow, 64), :],
                        qp_h[:, ds(qb * QB, QB)],
                        start=True, stop=False,
                    )
                    started = True
                p_kv = None
                if qb < N_QB - 1:
                    p_kv = pbig_pool.tile([64, 65], FP32, tag="big", name=f"pkv{h}")
                for cj in range(CPB):
                    c = qb * CPB + cj
                    nq = QB - cj * CH
                    qoff = qb * QB + cj * CH
                    # S^T for chunk c against remaining queries of the block
                    p_s = pbig_pool.tile([128, nq], FP32, tag="big")
                    nc.tensor.matmul(
                        p_s[:], kp_h[:, ds(c * CH, CH)], qp_h[:, ds(qoff, nq)],
                        start=True, stop=True,
                    )
                    s_t = ssb_pool.tile([128, nq], FP32R, tag="s")
                    nc.vector.tensor_mul(s_t[:, 0:CH], p_s[:, 0:CH], umask[:])
                    if nq > CH:
                        nc.scalar.copy(s_t[:, CH:nq], p_s[:, CH:nq])
                    # numerator (+denominator via ones column)
                    nc.tensor.matmul(
                        p_num[:, ds(cj * CH, nq)], vst[:, c, h, :], s_t[:],
                        start=not started, stop=(cj == CPB - 1),
                    )
                    started = True
                    # KV state update (skipped for the last block: never read)
                    if p_kv is not None:
                        p_t = pbig_pool.tile([128, 64], FP32, tag="big")
                        nc.tensor.transpose(
                            p_t[:], kp_h[:, ds(c * CH, CH)].bitcast(FP32), ident[ds(prow, 64), :]
                        )
                        kn_t = kn_pool.tile([128, 64], FP32, tag="kn")
                        nc.vector.tensor_copy(kn_t[:], p_t[:])
                        nc.tensor.matmul(
                            p_kv[:], kn_t[:], vst[:, c, h, :].bitcast(FP32),
                            start=(cj == 0), stop=(cj == CPB - 1),
                        )
                if p_kv is not None:
                    nc.vector.tensor_add(
                        kv_sb[h][ds(prow, 64), :], kv_sb[h][ds(prow, 64), :], p_kv[:]
                    )
                # divide: outT = num / (den + eps)
                den_t = den_pool.tile([1, QB], FP32, tag="den")
                nc.scalar.activation(den_t[:], p_num[ds(64, 1), :], AF.Ln, bias=0.0)
                rden = den_pool.tile([1, QB], FP32, tag="rden")
                nc.scalar.activation(rden[:], den_t[:], AF.Exp, scale=-1.0)
                bc_t = den_pool.tile([64, QB], FP32, tag="bc")
                nc.gpsimd.partition_broadcast(bc_t[:], rden[:])
                nc.vector.tensor_mul(
                    outt[mt][ds(prow, 64), ds(qb * QB, QB)], p_num[0:64, :], bc_t[:]
                )
            # O-projection for the 4 s-tiles of this query block
            for st in range(qb * CPB, (qb + 1) * CPB):
                y_t = yt_pool.tile([128, H_DIM], FP32, tag="y")
                for n in range(2):
                    p_o = pbig_pool.tile([128, 512], FP32, tag="big")
                    for ct in range(2):
                        nc.tensor.matmul(
                            p_o[:], outt[ct][:, ts(st, 128)],
                            wo_s[:, ct, ts(n, 512)],
                            start=(ct == 0), stop=(ct == 1),
                        )
                    nc.scalar.copy(y_t[:, ts(n, 512)], p_o[:])
                nc.sync.dma_start(y[ds(st * 128, 128), :], y_t[:])


_PROGRAM = None


def _get_program():
    global _PROGRAM
    if _PROGRAM is None:
        nc = bacc.Bacc("TRN2", target_bir_lowering=False, debug=False)
        with tile.TileContext(nc) as tc:
            _emit(tc)
        nc.compile()
        _PROGRAM = nc
    return _PROGRAM


def kernel(query, key, value, Wq, bq, Wk, bk, Wv, bv, Wo, bo, _trace=False):
    query, key, value = (np.asarray(a, np.float32) for a in (query, key, value))
    Wq, Wk, Wv, Wo = (np.asarray(a, np.float32) for a in (Wq, Wk, Wv, Wo))
    bq, bk, bv, bo = (np.asarray(a, np.float32) for a in (bq, bk, bv, bo))

    def wslice(W, g):  # (1024, 256) -> (128, 8, 256) contraction-chunked
        wt = W[g * O:(g + 1) * O].T  # (1024, 256)
        return np.ascontiguousarray(wt.reshape(KI, 128, O).transpose(1, 0, 2))

    in_maps = []
    for c in range(N_CORES):
        b, g = divmod(c, 4)
        sl = slice(g * O, (g + 1) * O)
        in_maps.append({
            "xqT": np.ascontiguousarray(query[b].T),
            "xkT": np.ascontiguousarray(key[b].T),
            "xvT": np.ascontiguousarray(value[b].T),
            "wq": wslice(Wq, g),
            "wk": wslice(Wk, g),
            "wv": wslice(Wv, g),
            "wo": np.ascontiguousarray(
                Wo[:, sl].T.reshape(2, 128, H_DIM).transpose(1, 0, 2)
            ),
            "bq": np.ascontiguousarray(bq[sl].reshape(2, 128).T),
            "bk": np.ascontiguousarray(bk[sl].reshape(2, 128).T),
            "bv": np.ascontiguousarray(bv[sl].reshape(1, O)),
        })

    nc = _get_program()
    res = run_bass_kernel_spmd(
        nc, in_maps, core_ids=list(range(N_CORES)), trace=_trace
    )
    out = np.empty((B, S, H_DIM), np.float32)
    for b in range(B):
        acc = res.results[4 * b]["y"].astype(np.float64)
        for g in range(1, 4):
            acc += res.results[4 * b + g]["y"]
        out[b] = (acc + bo).astype(np.float32)
    if _trace:
        kernel.last_result = res
    return out
